# revision 1
# baseline (speedup 1.0000x reference)
"""HGAT-ESM2 Bass kernel for 8 trn2 NeuronCores.

Key mathematical simplification: the reference's TypeAttention computes
seg_softmax(logit[d], d) where logit is per-dst-node -- values are constant
within each segment, so alpha_e = 1/indegree_r(dst_e), host-computable from
the indices alone.  The mul_W/mur_W/attn_w machinery cancels out entirely.

Remaining math per layer l (x = hom node features, table [N_hom, H]):
  hl = x @ Wl.T ; hr = x @ Wr.T
  s_e = sum_j leaky(hl[src_e, j] + hr[dst_e, j]) ; e_e = alpha_e * s_e
  a = seg_softmax(e, dst, N_hom) ; xo = segment_sum(a * x[src], dst)

Sharding: 61440 padded hom nodes = 8 shards x 7680 rows (5120 p + 2560 g per
core, interleaved renumbering).  Per layer one AllGather of the [x || hl]
table (61440 x 1024 f32); edge work is dst-block-sharded with uniform
per-core program structure (40 p-blocks x 7 chunks + 20 g-blocks x 8 chunks
in L1; 40 p-blocks x 7 chunks in L2 -- only pp/gp relations matter in the
last layer since only protein outputs survive).  Segment softmax + scatter
via one-hot matmuls on the PE; leaky+rowsum fused on ACT (Prelu+accum_out).
"""
import sys
sys.path.insert(0, "/opt/trn_rl_repo")
import numpy as np
import concourse.bass as bass
import concourse.bacc as bacc
import concourse.mybir as mybir
import concourse.tile as tile
from concourse.bass_utils import run_bass_kernel_spmd
from concourse.masks import make_identity

F32 = mybir.dt.float32
I32 = mybir.dt.int32
P = 128
NCORES = 8
NpR, NgR = 40000, 20000          # real node counts
PSH, GSH = 5120, 2560            # per-core padded p/g slots
SH = PSH + GSH                   # 7680 rows per shard
NH = SH * NCORES                 # 61440 padded hom nodes
PPC, GPC = NpR // NCORES, NgR // NCORES   # 5000 / 2500 real rows per core
NB_P, NB_G = PSH // P, GSH // P  # 40 / 20 blocks per core
C_P, C_G = 7, 8                  # chunk capacity per p-/g-block
T1 = NB_P * C_P + NB_G * C_G     # 440 chunk slots per core, layer 1
T2 = NB_P * C_P                  # 280 chunk slots per core, layer 2
PAD_ROW = NH - 1                 # a guaranteed-zero row in the xh tables
H, Dm, De, CLS = 512, 256, 1280, 1024
EPS = 1e-5
SLOPE = 0.2
AX = mybir.AxisListType.X
OP = mybir.AluOpType
AF = mybir.ActivationFunctionType


def _hom_ids(ids, is_g):
    ids = np.asarray(ids, np.int64)
    if is_g:
        c, r = ids // GPC, ids % GPC
        return c * SH + PSH + r
    c, r = ids // PPC, ids % PPC
    return c * SH + r


def _build_meta(rel_list, nb_g):
    """rel_list: list of (src_hom, dst_hom, w_alpha). Returns per-core dicts."""
    src = np.concatenate([r[0] for r in rel_list])
    dst = np.concatenate([r[1] for r in rel_list])
    w = np.concatenate([r[2] for r in rel_list]).astype(np.float32)
    core = dst // SH
    loc = dst % SH
    blk = loc // P
    dloc = (loc % P).astype(np.float32)
    nblk = NB_P + nb_g
    ncols = NB_P * C_P + nb_g * C_G
    metas = []
    for c in range(NCORES):
        sel = np.where(core == c)[0]
        g_idx = np.full((P, ncols), PAD_ROW, np.int32)
        d_loc = np.full((P, ncols), -1.0, np.float32)
        w_a = np.zeros((P, ncols), np.float32)
        order = sel[np.argsort(blk[sel], kind="stable")]
        bvals = blk[order]
        bounds = np.searchsorted(bvals, np.arange(nblk + 1))
        for b in range(nblk):
            e = order[bounds[b]:bounds[b + 1]]
            cap = C_P if b < NB_P else C_G
            col0 = b * C_P if b < NB_P else NB_P * C_P + (b - NB_P) * C_G
            k = len(e)
            assert k <= cap * P, f"block overflow: {k} > {cap * P}"
            lanes, chunks = np.arange(k) % P, np.arange(k) // P
            g_idx[lanes, col0 + chunks] = src[e]
            d_loc[lanes, col0 + chunks] = dloc[e]
            w_a[lanes, col0 + chunks] = w[e]
        # transposed one-hot per chunk: ohme[m, j*128 + e] = (d_loc[e, j] == m)
        ohme = (d_loc[None, :, :] == np.arange(P, dtype=np.float32)[:, None, None])
        ohme = ohme.astype(np.float32).transpose(0, 2, 1).reshape(P, ncols * P)
        metas.append(dict(g_idx=g_idx, d_loc=d_loc, w_a=w_a,
                          ohme=np.ascontiguousarray(ohme)))
    return metas


def _pad_rows(a, n):
    out = np.zeros((n, a.shape[1]), a.dtype)
    out[: a.shape[0]] = np.asarray(a, np.float32)
    return out


def _kn(W):
    """[K, N] -> [128, (K//128)*N] with chunk k of rows at columns [k*N:(k+1)*N]"""
    K, N = W.shape
    return np.ascontiguousarray(
        W.reshape(K // P, P, N).transpose(1, 0, 2).reshape(P, -1).astype(np.float32))


def _bc(row):
    return np.ascontiguousarray(
        np.tile(np.asarray(row, np.float32)[None, :], (P, 1)))


def build_program():
    nc = bacc.Bacc("TRN2", target_bir_lowering=False, debug=False,
                   num_devices=NCORES)

    def inp(name, shape, dt=F32):
        return nc.dram_tensor(name, list(shape), dt, kind="ExternalInput").ap()

    xpm = inp("xpm", [PSH, Dm]); xpe = inp("xpe", [PSH, De])
    xgm = inp("xgm", [GSH, Dm]); xge = inp("xge", [GSH, De])
    wnames = ["wm_p", "wm_g", "we_p", "we_g",
              "bfuse_p", "bfuse_g", "gbc_p", "gbc_g", "bbc_p", "bbc_g",
              "wl1_p", "wl1_g", "wr1_p", "wr1_g",
              "bl1_p", "bl1_g", "br1_p", "br1_g"]
    wshapes = {"wm_p": [P, (Dm // P) * (H // 2)], "wm_g": [P, (Dm // P) * (H // 2)],
               "we_p": [P, (De // P) * (H // 2)], "we_g": [P, (De // P) * (H // 2)],
               "bfuse_p": [P, H], "bfuse_g": [P, H],
               "gbc_p": [P, H], "gbc_g": [P, H],
               "bbc_p": [P, H], "bbc_g": [P, H],
               "wl1_p": [P, (H // P) * H], "wl1_g": [P, (H // P) * H],
               "wr1_p": [P, (H // P) * H], "wr1_g": [P, (H // P) * H],
               "bl1_p": [P, H], "bl1_g": [P, H],
               "br1_p": [P, H], "br1_g": [P, H]}
    wap = {n: inp(n, wshapes[n]) for n in wnames}
    wl2 = inp("wl2", [P, (H // P) * H]); wr2 = inp("wr2", [P, (H // P) * H])
    wcls = inp("wcls", [P, (H // P) * CLS]); bcls_bc = inp("bcls_bc", [P, CLS])
    iota_in = inp("iota_in", [P, P])
    gi1 = inp("gi1", [P, T1], I32); dl1 = inp("dl1", [P, T1]); wa1 = inp("wa1", [P, T1])
    om1 = inp("om1", [P, T1 * P])
    gi2 = inp("gi2", [P, T2], I32); dl2 = inp("dl2", [P, T2]); wa2 = inp("wa2", [P, T2])
    om2 = inp("om2", [P, T2 * P])
    out_d = nc.dram_tensor("out", [PSH, CLS], F32, kind="ExternalOutput").ap()

    with tile.TileContext(nc, num_cores=NCORES) as tc:
        with (
            tc.tile_pool(name="dram", bufs=1, space="DRAM") as dram,
            tc.tile_pool(name="const", bufs=1) as cpool,
        ):
            xh1_sh = dram.tile([SH, 2 * H], F32)
            xh1_full = dram.tile([NH, 2 * H], F32, addr_space="Shared")
            xh2_sh = dram.tile([SH, 2 * H], F32)
            xh2_full = dram.tile([NH, 2 * H], F32, addr_space="Shared")
            hr1_tab = dram.tile([SH, H], F32)

            iota = cpool.tile([P, P], F32)
            nc.sync.dma_start(out=iota[:], in_=iota_in[:])
            ident = cpool.tile([P, P], F32)
            make_identity(nc, ident[:])

            meta_t = {}
            for name, ap, dt in (("gi1", gi1, I32), ("dl1", dl1, F32),
                                 ("wa1", wa1, F32), ("gi2", gi2, I32),
                                 ("dl2", dl2, F32), ("wa2", wa2, F32)):
                t = cpool.tile(list(ap.shape), dt, name=f"m_{name}")
                nc.sync.dma_start(out=t[:], in_=ap[:])
                meta_t[name] = t

            def transposes(src_sb, psum_pool, sbuf_pool, nchunks, tag):
                """PE-transpose [128, n*128] -> list of [128,128] sbuf tiles."""
                outs = []
                for k in range(nchunks):
                    ps = psum_pool.tile([P, P], F32, name=f"tp_{tag}",
                                        tag="small", space="PSUM")
                    nc.tensor.transpose(out=ps[:], in_=src_sb[:, k * P:(k + 1) * P],
                                        identity=ident[:])
                    sb = sbuf_pool.tile([P, P], F32, name=f"ts_{tag}{k}",
                                        tag=f"ts_{tag}{k}")
                    nc.vector.tensor_copy(out=sb[:], in_=ps[:])
                    outs.append(sb)
                return outs

            def layernorm(x_sb, pool, tag):
                """returns xn tile = (x - mean)/sqrt(var+eps), per-row stats."""
                s1 = pool.tile([P, 1], F32, name=f"{tag}s1", tag=f"{tag}s1")
                nc.vector.reduce_sum(out=s1[:], in_=x_sb[:], axis=AX)
                sq = pool.tile([P, H], F32, name=f"{tag}sq", tag=f"{tag}sq")
                ssq = pool.tile([P, 1], F32, name=f"{tag}ssq", tag=f"{tag}ssq")
                nc.scalar.activation(out=sq[:], in_=x_sb[:], func=AF.Square,
                                     accum_out=ssq[:])
                mu = pool.tile([P, 1], F32, name=f"{tag}mu", tag=f"{tag}mu")
                nc.vector.tensor_scalar_mul(out=mu[:], in0=s1[:], scalar1=1.0 / H)
                var = pool.tile([P, 1], F32, name=f"{tag}var", tag=f"{tag}var")
                nc.vector.tensor_scalar_mul(out=var[:], in0=ssq[:], scalar1=1.0 / H)
                mu2 = pool.tile([P, 1], F32, name=f"{tag}mu2", tag=f"{tag}mu2")
                nc.vector.tensor_tensor(out=mu2[:], in0=mu[:], in1=mu[:], op=OP.mult)
                nc.vector.tensor_tensor(out=var[:], in0=var[:], in1=mu2[:],
                                        op=OP.subtract)
                nc.vector.tensor_scalar_add(out=var[:], in0=var[:], scalar1=EPS)
                rv = pool.tile([P, 1], F32, name=f"{tag}rv", tag=f"{tag}rv")
                nc.vector.reciprocal(out=rv[:], in_=var[:])
                rstd = pool.tile([P, 1], F32, name=f"{tag}rstd", tag=f"{tag}rstd")
                nc.scalar.activation(out=rstd[:], in_=rv[:], func=AF.Sqrt)
                xn = pool.tile([P, H], F32, name=f"{tag}xn", tag=f"{tag}xn")
                nc.vector.tensor_scalar(out=xn[:], in0=x_sb[:], scalar1=mu[:],
                                        scalar2=rstd[:], op0=OP.subtract,
                                        op1=OP.mult)
                return xn

            # ================= FUSE + layer-1 hl/hr =================
            with (
                tc.tile_pool(name="fw", bufs=1) as fw,
                tc.tile_pool(name="fsb", bufs=2) as fsb,
                tc.tile_pool(name="fps", bufs=2, space="PSUM") as fps,
            ):
                W = {}
                for n in wnames:
                    t = fw.tile(list(wshapes[n]), F32, name=f"c_{n}")
                    nc.sync.dma_start(out=t[:], in_=wap[n][:])
                    W[n] = t
                zero_row = fw.tile([P, 2 * H], F32)
                nc.vector.memset(zero_row[:], 0.0)

                for b in range(NB_P + NB_G):
                    sfx = "p" if b < NB_P else "g"
                    r0 = b * P if b < NB_P else (b - NB_P) * P
                    xm_src, xe_src = (xpm, xpe) if b < NB_P else (xgm, xge)
                    xm = fsb.tile([P, Dm], F32, tag="xm", name="xm")
                    xe = fsb.tile([P, De], F32, tag="xe", name="xe")
                    nc.sync.dma_start(out=xm[:], in_=xm_src[r0:r0 + P, :])
                    nc.sync.dma_start(out=xe[:], in_=xe_src[r0:r0 + P, :])
                    xmT = transposes(xm, fps, fsb, Dm // P, "xm")
                    xeT = transposes(xe, fps, fsb, De // P, "xe")
                    hps = fps.tile([P, H], F32, tag="hps", space="PSUM", name="hps")
                    for k in range(Dm // P):
                        nc.tensor.matmul(out=hps[:, :H // 2], lhsT=xmT[k][:],
                                         rhs=W[f"wm_{sfx}"][:, k * (H // 2):(k + 1) * (H // 2)],
                                         start=(k == 0), stop=(k == Dm // P - 1))
                    for k in range(De // P):
                        nc.tensor.matmul(out=hps[:, H // 2:], lhsT=xeT[k][:],
                                         rhs=W[f"we_{sfx}"][:, k * (H // 2):(k + 1) * (H // 2)],
                                         start=(k == 0), stop=(k == De // P - 1))
                    cat = fsb.tile([P, H], F32, tag="cat", name="cat")
                    nc.vector.tensor_tensor(out=cat[:], in0=hps[:],
                                            in1=W[f"bfuse_{sfx}"][:], op=OP.add)
                    xn = layernorm(cat, fsb, "f_")
                    h1 = fsb.tile([P, H], F32, tag="h1", name="h1")
                    nc.vector.tensor_tensor(out=h1[:], in0=xn[:],
                                            in1=W[f"gbc_{sfx}"][:], op=OP.mult)
                    nc.vector.tensor_tensor(out=h1[:], in0=h1[:],
                                            in1=W[f"bbc_{sfx}"][:], op=OP.add)
                    nc.sync.dma_start(out=xh1_sh[b * P:(b + 1) * P, 0:H], in_=h1[:])
                    xnT = transposes(xn, fps, fsb, H // P, "xn")
                    for wnm, bnm, dst_ap in (
                        ("wl1", "bl1", xh1_sh[b * P:(b + 1) * P, H:2 * H]),
                        ("wr1", "br1", hr1_tab[b * P:(b + 1) * P, :]),
                    ):
                        ps = fps.tile([P, H], F32, tag="hlr", space="PSUM", name="hlr")
                        for k in range(H // P):
                            nc.tensor.matmul(out=ps[:], lhsT=xnT[k][:],
                                             rhs=W[f"{wnm}_{sfx}"][:, k * H:(k + 1) * H],
                                             start=(k == 0), stop=(k == H // P - 1))
                        sb = fsb.tile([P, H], F32, tag="hlr_sb", name="hlr_sb")
                        nc.vector.tensor_tensor(out=sb[:], in0=ps[:],
                                                in1=W[f"{bnm}_{sfx}"][:], op=OP.add)
                        nc.sync.dma_start(out=dst_ap, in_=sb[:])
                # zero the pad rows (so PAD_ROW gathers and pad dst reads are clean)
                for lo, hi in ((PPC, PSH), (PSH + GPC, SH)):
                    r = lo
                    while r < hi:
                        n = min(P, hi - r)
                        nc.sync.dma_start(out=xh1_sh[r:r + n, :], in_=zero_row[:n, :])
                        nc.sync.dma_start(out=xh2_sh[r:r + n, :], in_=zero_row[:n, :])
                        nc.sync.dma_start(out=hr1_tab[r:r + n, :],
                                          in_=zero_row[:n, :H])
                        r += n

            nc.gpsimd.collective_compute(
                "AllGather", OP.bypass, replica_groups=[list(range(NCORES))],
                ins=[xh1_sh.opt()], outs=[xh1_full.opt()])

            # ================= edge phase (both layers) =================
            def edge_phase(layer, xh_full, nblocks, caps, gi, dl, wa, om_ap,
                           hr_fn, epilogue):
                with (
                    tc.tile_pool(name=f"esb{layer}", bufs=2) as esb,
                    tc.tile_pool(name=f"exs{layer}", bufs=2) as exs,
                    tc.tile_pool(name=f"eps{layer}", bufs=2, space="PSUM") as eps,
                    tc.tile_pool(name=f"exo{layer}", bufs=1, space="PSUM") as exo,
                ):
                    col0 = 0
                    for b in range(nblocks):
                        C = caps[b]
                        hr_sb = hr_fn(b, esb, eps)
                        xs = exs.tile([P, C_G * 2 * H], F32, tag="xs", name="xs")
                        for j in range(C):
                            nc.gpsimd.indirect_dma_start(
                                out=xs[:, j * 2 * H:(j + 1) * 2 * H],
                                out_offset=None, in_=xh_full[:],
                                in_offset=bass.IndirectOffsetOnAxis(
                                    ap=meta_t[gi][:, col0 + j:col0 + j + 1], axis=0))
                        om_blk = esb.tile([P, C_G * P], F32, tag="om", name="om_blk")
                        nc.sync.dma_start(out=om_blk[:, :C * P],
                                          in_=om_ap[:, col0 * P:(col0 + C) * P])
                        e_all = esb.tile([P, C_G], F32, tag="e_all", name="e_all")
                        u_all = esb.tile([P, C_G], F32, tag="u_all", name="u_all")
                        oh_all = esb.tile([P, C_G * P], F32, tag="oh_all", name="oh_all")
                        kmax = esb.tile([P, 1], F32, tag="kmax", name="kmax")
                        nc.vector.memset(kmax[:], 0.0)
                        for j in range(C):
                            he = eps.tile([P, H], F32, tag="big", space="PSUM", name="he")
                            nc.tensor.matmul(out=he[:], lhsT=om_blk[:, j * P:(j + 1) * P],
                                             rhs=hr_sb[:], start=True, stop=True)
                            t = esb.tile([P, H], F32, tag="t", name="t")
                            nc.vector.tensor_tensor(
                                out=t[:], in0=xs[:, j * 2 * H + H:(j + 1) * 2 * H],
                                in1=he[:], op=OP.add)
                            lk = esb.tile([P, H], F32, tag="lk", name="lk")
                            s_col = esb.tile([P, 1], F32, tag="s_col", name="s_col")
                            nc.scalar.activation(out=lk[:], in_=t[:], func=AF.Prelu,
                                                 alpha=SLOPE, accum_out=s_col[:])
                            nc.vector.tensor_tensor(
                                out=e_all[:, j:j + 1], in0=s_col[:],
                                in1=meta_t[wa][:, col0 + j:col0 + j + 1], op=OP.mult)
                            nc.vector.tensor_tensor(
                                out=oh_all[:, j * P:(j + 1) * P],
                                in0=meta_t[dl][:, col0 + j:col0 + j + 1].to_broadcast([P, P]),
                                in1=iota[:], op=OP.is_equal)
                            m2 = esb.tile([P, P], F32, tag="m2", name="m2")
                            nc.vector.tensor_scalar(
                                out=m2[:], in0=oh_all[:, j * P:(j + 1) * P],
                                scalar1=e_all[:, j:j + 1], scalar2=None, op0=OP.mult)
                            m2T = eps.tile([P, P], F32, tag="small", space="PSUM",
                                           name="m2T")
                            nc.tensor.transpose(out=m2T[:], in_=m2[:], identity=ident[:])
                            cmax = esb.tile([P, 1], F32, tag="cmax", name="cmax")
                            nc.vector.tensor_reduce(out=cmax[:], in_=m2T[:], axis=AX,
                                                    op=OP.max)
                            nc.vector.tensor_tensor(out=kmax[:], in0=kmax[:],
                                                    in1=cmax[:], op=OP.max)
                        negk = esb.tile([P, 1], F32, tag="negk", name="negk")
                        nc.vector.tensor_scalar_mul(out=negk[:], in0=kmax[:],
                                                    scalar1=-1.0)
                        den = exo.tile([P, 1], F32, tag="den", space="PSUM", name="den")
                        for j in range(C):
                            nke = eps.tile([P, 1], F32, tag="small", space="PSUM",
                                           name="nke")
                            nc.tensor.matmul(out=nke[:], lhsT=om_blk[:, j * P:(j + 1) * P],
                                             rhs=negk[:], start=True, stop=True)
                            nke_sb = esb.tile([P, 1], F32, tag="nke_sb", name="nke_sb")
                            nc.vector.tensor_copy(out=nke_sb[:], in_=nke[:])
                            nc.scalar.activation(out=u_all[:, j:j + 1],
                                                 in_=e_all[:, j:j + 1], func=AF.Exp,
                                                 bias=nke_sb[:])
                            nc.tensor.matmul(out=den[:], lhsT=oh_all[:, j * P:(j + 1) * P],
                                             rhs=u_all[:, j:j + 1],
                                             start=(j == 0), stop=(j == C - 1))
                        den_sb = esb.tile([P, 1], F32, tag="den_sb", name="den_sb")
                        nc.vector.tensor_scalar_max(out=den_sb[:], in0=den[:],
                                                    scalar1=1e-30)
                        rden = esb.tile([P, 1], F32, tag="rden", name="rden")
                        nc.vector.reciprocal(out=rden[:], in_=den_sb[:])
                        xo_ps = exo.tile([P, H], F32, tag="xo", space="PSUM", name="xo")
                        for j in range(C):
                            rde = eps.tile([P, 1], F32, tag="small", space="PSUM",
                                           name="rde")
                            nc.tensor.matmul(out=rde[:], lhsT=om_blk[:, j * P:(j + 1) * P],
                                             rhs=rden[:], start=True, stop=True)
                            a_col = esb.tile([P, 1], F32, tag="a_col", name="a_col")
                            nc.vector.tensor_tensor(out=a_col[:], in0=u_all[:, j:j + 1],
                                                    in1=rde[:], op=OP.mult)
                            oha = esb.tile([P, P], F32, tag="oha", name="oha")
                            nc.vector.tensor_scalar(out=oha[:],
                                                    in0=oh_all[:, j * P:(j + 1) * P],
                                                    scalar1=a_col[:], scalar2=None,
                                                    op0=OP.mult)
                            nc.tensor.matmul(out=xo_ps[:], lhsT=oha[:],
                                             rhs=xs[:, j * 2 * H:j * 2 * H + H],
                                             start=(j == 0), stop=(j == C - 1))
                        xo_sb = esb.tile([P, H], F32, tag="xo_sb", name="xo_sb")
                        nc.vector.tensor_copy(out=xo_sb[:], in_=xo_ps[:])
                        epilogue(b, xo_sb, esb, eps)
                        col0 += C

            with tc.tile_pool(name="l2w", bufs=1) as l2w:
                wl2_t = l2w.tile([P, (H // P) * H], F32, name="wl2_t")
                nc.sync.dma_start(out=wl2_t[:], in_=wl2[:])
                wr2_t = l2w.tile([P, (H // P) * H], F32, name="wr2_t")
                nc.sync.dma_start(out=wr2_t[:], in_=wr2[:])

                # ---- layer 1 ----
                def hr1_fn(b, esb, eps):
                    hr_sb = esb.tile([P, H], F32, tag="hr_sb", name="hr_sb")
                    nc.sync.dma_start(out=hr_sb[:], in_=hr1_tab[b * P:(b + 1) * P, :])
                    return hr_sb

                def epi1(b, xo_sb, esb, eps):
                    nc.sync.dma_start(out=xh2_sh[b * P:(b + 1) * P, 0:H], in_=xo_sb[:])
                    xoT = transposes(xo_sb, eps, esb, H // P, "xoT")
                    ps = eps.tile([P, H], F32, tag="big", space="PSUM", name="hl2ps")
                    for k in range(H // P):
                        nc.tensor.matmul(out=ps[:], lhsT=xoT[k][:],
                                         rhs=wl2_t[:, k * H:(k + 1) * H],
                                         start=(k == 0), stop=(k == H // P - 1))
                    sb = esb.tile([P, H], F32, tag="hl2sb", name="hl2sb")
                    nc.vector.tensor_copy(out=sb[:], in_=ps[:])
                    nc.sync.dma_start(out=xh2_sh[b * P:(b + 1) * P, H:2 * H], in_=sb[:])

                edge_phase(1, xh1_full, NB_P + NB_G, [C_P] * NB_P + [C_G] * NB_G,
                           "gi1", "dl1", "wa1", om1, hr1_fn, epi1)

                nc.gpsimd.collective_compute(
                    "AllGather", OP.bypass, replica_groups=[list(range(NCORES))],
                    ins=[xh2_sh.opt()], outs=[xh2_full.opt()])

                # ---- layer 2 + head LN + cls ----
                with tc.tile_pool(name="clsw", bufs=1) as clsw:
                    wcls_t = clsw.tile([P, (H // P) * CLS], F32, name="wcls_t")
                    nc.sync.dma_start(out=wcls_t[:], in_=wcls[:])
                    bcls_t = clsw.tile([P, CLS], F32, name="bcls_t")
                    nc.sync.dma_start(out=bcls_t[:], in_=bcls_bc[:])

                    def hr2_fn(b, esb, eps):
                        xr = esb.tile([P, H], F32, tag="xr", name="xr")
                        nc.sync.dma_start(out=xr[:],
                                          in_=xh2_sh[b * P:(b + 1) * P, 0:H])
                        xrT = transposes(xr, eps, esb, H // P, "xrT")
                        ps = eps.tile([P, H], F32, tag="big", space="PSUM",
                                      name="hr2ps")
                        for k in range(H // P):
                            nc.tensor.matmul(out=ps[:], lhsT=xrT[k][:],
                                             rhs=wr2_t[:, k * H:(k + 1) * H],
                                             start=(k == 0), stop=(k == H // P - 1))
                        hr_sb = esb.tile([P, H], F32, tag="hr_sb", name="hr_sb")
                        nc.vector.tensor_copy(out=hr_sb[:], in_=ps[:])
                        return hr_sb

                    def epi2(b, xo_sb, esb, eps):
                        xn = layernorm(xo_sb, esb, "c_")
                        xnT = transposes(xn, eps, esb, H // P, "c_xnT")
                        for half in range(2):
                            ps = eps.tile([P, H], F32, tag="big", space="PSUM",
                                          name="clsps")
                            for k in range(H // P):
                                nc.tensor.matmul(
                                    out=ps[:], lhsT=xnT[k][:],
                                    rhs=wcls_t[:, k * CLS + half * H:
                                               k * CLS + (half + 1) * H],
                                    start=(k == 0), stop=(k == H // P - 1))
                            ob = esb.tile([P, H], F32, tag="c_ob", name="c_ob")
                            nc.vector.tensor_tensor(
                                out=ob[:], in0=ps[:],
                                in1=bcls_t[:, half * H:(half + 1) * H], op=OP.add)
                            nc.sync.dma_start(
                                out=out_d[b * P:(b + 1) * P,
                                          half * H:(half + 1) * H],
                                in_=ob[:])

                    edge_phase(2, xh2_full, NB_P, [C_P] * NB_P,
                               "gi2", "dl2", "wa2", om2, hr2_fn, epi2)

    nc.compile()
    return nc


def kernel(_run_kwargs=None, **inputs):
    run_kwargs = _run_kwargs or {}
    inp = {k: np.asarray(v) for k, v in inputs.items()}

    def rel_arrays(st, dt, s, d):
        nt = NpR if dt == 0 else NgR
        ideg = np.bincount(d, minlength=nt)
        w = (1.0 / np.maximum(ideg[d], 1)).astype(np.float32)
        return (_hom_ids(s, st == 1), _hom_ids(d, dt == 1), w)

    rels = [(0, 0, inp["pp_src"], inp["pp_dst"]),
            (0, 1, inp["pg_src"], inp["pg_dst"]),
            (1, 0, inp["gp_src"], inp["gp_dst"]),
            (1, 1, inp["gg_src"], inp["gg_dst"])]
    rel1 = [rel_arrays(*r) for r in rels]
    meta1 = _build_meta(rel1, NB_G)
    rel2 = [rel_arrays(*r) for r in rels if r[1] == 0]
    meta2 = _build_meta(rel2, 0)

    lnf_p_g, lnf_p_b = inp["lnf_p_g"], inp["lnf_p_b"]
    lnf_g_g, lnf_g_b = inp["lnf_g_g"], inp["lnf_g_b"]
    wl1 = np.ascontiguousarray(inp["node_Wl"][0].T, dtype=np.float32)
    wr1 = np.ascontiguousarray(inp["node_Wr"][0].T, dtype=np.float32)
    wl2 = np.ascontiguousarray(inp["node_Wl"][1].T, dtype=np.float32)
    wr2 = np.ascontiguousarray(inp["node_Wr"][1].T, dtype=np.float32)
    hg, hb = inp["head_ln_g"], inp["head_ln_b"]
    wcls_T = np.ascontiguousarray(inp["Wcls"].T, dtype=np.float32)

    common = dict(
        wm_p=_kn(np.ascontiguousarray(inp["Wmsa_p"].T, np.float32)),
        wm_g=_kn(np.ascontiguousarray(inp["Wmsa_g"].T, np.float32)),
        we_p=_kn(np.ascontiguousarray(inp["Wesm_p"].T, np.float32)),
        we_g=_kn(np.ascontiguousarray(inp["Wesm_g"].T, np.float32)),
        bfuse_p=_bc(np.concatenate([inp["bmsa_p"], np.zeros(H // 2)])),
        bfuse_g=_bc(np.concatenate([inp["bmsa_g"], np.zeros(H // 2)])),
        gbc_p=_bc(lnf_p_g), bbc_p=_bc(lnf_p_b),
        gbc_g=_bc(lnf_g_g), bbc_g=_bc(lnf_g_b),
        wl1_p=_kn(lnf_p_g[:, None] * wl1),
        wl1_g=_kn(lnf_g_g[:, None] * wl1),
        wr1_p=_kn(lnf_p_g[:, None] * wr1),
        wr1_g=_kn(lnf_g_g[:, None] * wr1),
        bl1_p=_bc(lnf_p_b @ wl1), bl1_g=_bc(lnf_g_b @ wl1),
        br1_p=_bc(lnf_p_b @ wr1), br1_g=_bc(lnf_g_b @ wr1),
        wl2=_kn(wl2), wr2=_kn(wr2),
        wcls=_kn(hg[:, None] * wcls_T),
        bcls_bc=_bc(np.asarray(inp["bcls"]) + hb @ wcls_T),
        iota_in=np.ascontiguousarray(
            np.tile(np.arange(P, dtype=np.float32)[None, :], (P, 1))),
    )

    in_maps = []
    for c in range(NCORES):
        m = dict(common)
        m["xpm"] = _pad_rows(inp["xp_msa"][c * PPC:(c + 1) * PPC], PSH)
        m["xpe"] = _pad_rows(inp["xp_esm"][c * PPC:(c + 1) * PPC], PSH)
        m["xgm"] = _pad_rows(inp["xg_msa"][c * GPC:(c + 1) * GPC], GSH)
        m["xge"] = _pad_rows(inp["xg_esm"][c * GPC:(c + 1) * GPC], GSH)
        m["gi1"], m["dl1"], m["wa1"], m["om1"] = (
            meta1[c]["g_idx"], meta1[c]["d_loc"], meta1[c]["w_a"], meta1[c]["ohme"])
        m["gi2"], m["dl2"], m["wa2"], m["om2"] = (
            meta2[c]["g_idx"], meta2[c]["d_loc"], meta2[c]["w_a"], meta2[c]["ohme"])
        in_maps.append(m)

    nc = build_program()
    res = run_bass_kernel_spmd(nc, in_maps, core_ids=list(range(NCORES)),
                               **run_kwargs)
    out = np.concatenate([res.results[c]["out"][:PPC] for c in range(NCORES)], 0)
    if run_kwargs:
        return np.ascontiguousarray(out, np.float32), res
    return np.ascontiguousarray(out, np.float32)


if __name__ == "__main__":
    import reference
    inputs = {k: np.asarray(v) for k, v in reference.setup_inputs().items()}
    got = kernel(**inputs)
    print("kernel output", got.shape, got.dtype, "finite:", np.isfinite(got).all())



# revision 5
# speedup vs baseline: 2.1886x; 2.1886x over previous
"""HGAT-ESM2 Bass kernel for 8 trn2 NeuronCores (v2, bf16).

Math simplification (unchanged from v1): TypeAttention's seg_softmax over
per-dst-node logits is constant within each segment, so alpha_e =
1/indegree_r(dst_e) -- host-computable from indices alone.

Per layer l on the homogeneous table x [N_hom, H]:
  hl = x @ Wl.T ; hr = x @ Wr.T
  s_e = sum_j leaky(hl[src_e, j] + hr[dst_e, j]) ; e_e = alpha_e * s_e
  a = seg_softmax(e, dst, N_hom) ; xo = segment_sum(a * x[src], dst)

v2 changes vs v1:
  * bf16 matmul operands / tables / gathers / one-hot masks (PE fp32 runs at
    4 cycles/row vs bf16 1; tables+gathers halve DMA; AllGather halves).
    Stats, softmax intermediates and PSUM accumulation stay fp32.
  * fuse-phase inputs arrive pre-transposed from the host (no PE transposes
    or PSUM->SBUF copies for xm/xe).
  * hr2 = xo @ Wr2 computed in L1's epilogue (xoT already available there).
  * AllGathers split into 4 row-quarters, each into its own Shared DRAM tile
    (the Tile framework allows only one writer per Shared tile).  Edge-phase
    gather chunks are quarter-pure (2 chunk slots per quarter per block), so
    each chunk reads one quarter tile; quarter AGs are issued as soon as
    their producer blocks finish and overlap with fuse / L1 compute, and the
    edge phase can begin gathering quarter q as soon as AG q lands.
  * per-chunk [128,1] vector ops batched into per-block [128,C] ops.
"""
import sys
sys.path.insert(0, "/opt/trn_rl_repo")
import numpy as np
import ml_dtypes
import concourse.bass as bass
import concourse.bacc as bacc
import concourse.mybir as mybir
import concourse.tile as tile
from concourse.bass_utils import run_bass_kernel_spmd
from concourse.masks import make_identity

F32 = mybir.dt.float32
BF16 = mybir.dt.bfloat16
I32 = mybir.dt.int32
NPBF = ml_dtypes.bfloat16
P = 128
NCORES = 8
NpR, NgR = 40000, 20000          # real node counts
PSH, GSH = 5120, 2560            # per-core padded p/g slots
SH = PSH + GSH                   # 7680 rows per shard
QROWS = SH // 4                  # 1920 rows per AllGather quarter
QFULL = NCORES * QROWS           # 15360 rows per gathered quarter tile
PPC, GPC = NpR // NCORES, NgR // NCORES   # 5000 / 2500 real rows per core
NB_P, NB_G = PSH // P, GSH // P  # 40 / 20 blocks per core
CQ = 2                           # chunk slots per (block, quarter)
CB = 4 * CQ                      # 8 chunk slots per block
T1 = (NB_P + NB_G) * CB          # 480 chunk slots per core, layer 1
T2 = NB_P * CB                   # 320 chunk slots per core, layer 2
H, Dm, De, CLS = 512, 256, 1280, 1024
KM, KE, KH = Dm // P, De // P, H // P     # 2 / 10 / 4 contraction chunks
EPS = 1e-5
SLOPE = 0.2
AX = mybir.AxisListType.X
OP = mybir.AluOpType
AF = mybir.ActivationFunctionType


def _src_glob(ids, is_g):
    """node ids -> quarter-major global table row."""
    ids = np.asarray(ids, np.int64)
    if is_g:
        c, loc = ids // GPC, PSH + ids % GPC
    else:
        c, loc = ids // PPC, ids % PPC
    q, r = loc // QROWS, loc % QROWS
    return q * QFULL + c * QROWS + r


def _dst_local(ids, is_g):
    ids = np.asarray(ids, np.int64)
    if is_g:
        return ids // GPC, PSH + ids % GPC
    return ids // PPC, ids % PPC


def _build_meta(rel_list, nb_g):
    """rel_list: list of (src_glob, dst_core, dst_loc, w_alpha).

    Chunks are quarter-pure: block b's chunk slots [q*CQ:(q+1)*CQ] hold only
    edges whose src row lives in quarter q; gi stores the in-quarter row.
    """
    src = np.concatenate([r[0] for r in rel_list])
    core = np.concatenate([r[1] for r in rel_list])
    loc = np.concatenate([r[2] for r in rel_list])
    w = np.concatenate([r[3] for r in rel_list]).astype(np.float32)
    blk = loc // P
    dloc = (loc % P).astype(np.float32)
    srcq = src // QFULL
    srcl = (src % QFULL).astype(np.int32)
    nblk = NB_P + nb_g
    ncols = nblk * CB
    metas = []
    for c in range(NCORES):
        sel = np.where(core == c)[0]
        g_idx = np.zeros((P, ncols), np.int32)
        d_loc = np.full((P, ncols), -1.0, np.float32)
        w_a = np.zeros((P, ncols), np.float32)
        key = blk[sel] * 4 + srcq[sel]
        order = sel[np.argsort(key, kind="stable")]
        kvals = blk[order] * 4 + srcq[order]
        bounds = np.searchsorted(kvals, np.arange(nblk * 4 + 1))
        for bq in range(nblk * 4):
            e = order[bounds[bq]:bounds[bq + 1]]
            b, q = bq // 4, bq % 4
            col0 = b * CB + q * CQ
            k = len(e)
            assert k <= CQ * P, f"block/quarter overflow: {k} > {CQ * P}"
            lanes, chunks = np.arange(k) % P, np.arange(k) // P
            g_idx[lanes, col0 + chunks] = srcl[e]
            d_loc[lanes, col0 + chunks] = dloc[e]
            w_a[lanes, col0 + chunks] = w[e]
        # transposed one-hot per chunk: ohme[m, j*128 + e] = (d_loc[e, j] == m)
        ohme = (d_loc[None, :, :] == np.arange(P, dtype=np.float32)[:, None, None])
        ohme = ohme.astype(np.float32).transpose(0, 2, 1).reshape(P, ncols * P)
        metas.append(dict(g_idx=g_idx, d_loc=d_loc, w_a=w_a,
                          ohme=np.ascontiguousarray(ohme.astype(NPBF))))
    return metas


def _kn(W):
    """[K, N] -> bf16 [128, (K//128)*N], chunk k of rows at cols [k*N:(k+1)*N]"""
    K, N = W.shape
    return np.ascontiguousarray(
        W.reshape(K // P, P, N).transpose(1, 0, 2).reshape(P, -1).astype(NPBF))


def _bc(row):
    return np.ascontiguousarray(
        np.tile(np.asarray(row, np.float32)[None, :], (P, 1)))


def _xT(x, nblk, kch):
    """[rows, D] -> bf16 [128, nblk*kch*128]; block b chunk k at
    cols [(b*kch+k)*128 : ...], T[d, (b*kch+k)*128+i] = x[b*128+i, k*128+d]."""
    A = np.zeros((nblk * P, kch * P), np.float32)
    A[: x.shape[0]] = np.asarray(x, np.float32)
    A = A.reshape(nblk, P, kch, P).transpose(3, 0, 2, 1).reshape(P, -1)
    return np.ascontiguousarray(A.astype(NPBF))


def build_program():
    nc = bacc.Bacc("TRN2", target_bir_lowering=False, debug=False,
                   num_devices=NCORES)

    def inp(name, shape, dt=BF16):
        return nc.dram_tensor(name, list(shape), dt, kind="ExternalInput").ap()

    xmt_p = inp("xmt_p", [P, NB_P * KM * P]); xet_p = inp("xet_p", [P, NB_P * KE * P])
    xmt_g = inp("xmt_g", [P, NB_G * KM * P]); xet_g = inp("xet_g", [P, NB_G * KE * P])
    wnames = ["wm_p", "wm_g", "we_p", "we_g",
              "wl1_p", "wl1_g", "wr1_p", "wr1_g"]
    wshapes = {"wm_p": [P, KM * (H // 2)], "wm_g": [P, KM * (H // 2)],
               "we_p": [P, KE * (H // 2)], "we_g": [P, KE * (H // 2)],
               "wl1_p": [P, KH * H], "wl1_g": [P, KH * H],
               "wr1_p": [P, KH * H], "wr1_g": [P, KH * H]}
    bnames = ["bfuse_p", "bfuse_g", "gbc_p", "gbc_g", "bbc_p", "bbc_g",
              "bl1_p", "bl1_g", "br1_p", "br1_g"]
    wap = {n: inp(n, wshapes[n]) for n in wnames}
    bap = {n: inp(n, [P, H], F32) for n in bnames}
    wl2 = inp("wl2", [P, KH * H]); wr2 = inp("wr2", [P, KH * H])
    wcls = inp("wcls", [P, KH * CLS]); bcls_bc = inp("bcls_bc", [P, CLS], F32)
    iota_in = inp("iota_in", [P, P], F32)
    gi1 = inp("gi1", [P, T1], I32); dl1 = inp("dl1", [P, T1], F32)
    wa1 = inp("wa1", [P, T1], F32)
    om1 = inp("om1", [P, T1 * P])
    gi2 = inp("gi2", [P, T2], I32); dl2 = inp("dl2", [P, T2], F32)
    wa2 = inp("wa2", [P, T2], F32)
    om2 = inp("om2", [P, T2 * P])
    out_d = nc.dram_tensor("out", [PSH, CLS], F32, kind="ExternalOutput").ap()

    with tile.TileContext(nc, num_cores=NCORES) as tc:
        with (
            tc.tile_pool(name="dram", bufs=1, space="DRAM") as dram,
            tc.tile_pool(name="const", bufs=1) as cpool,
        ):
            xh1_q = [dram.tile([QROWS, 2 * H], BF16, name=f"xh1_q{q}")
                     for q in range(4)]
            xh2_q = [dram.tile([QROWS, 2 * H], BF16, name=f"xh2_q{q}")
                     for q in range(4)]
            xh1_f = [dram.tile([QFULL, 2 * H], BF16, addr_space="Shared",
                               name=f"xh1_f{q}") for q in range(4)]
            xh2_f = [dram.tile([QFULL, 2 * H], BF16, addr_space="Shared",
                               name=f"xh2_f{q}") for q in range(4)]
            hr1_tab = dram.tile([SH, H], BF16)
            hr2_tab = dram.tile([PSH, H], BF16)

            iota = cpool.tile([P, P], F32)
            nc.sync.dma_start(out=iota[:], in_=iota_in[:])
            ident = cpool.tile([P, P], BF16)
            make_identity(nc, ident[:])

            meta_t = {}
            for name, ap, dt in (("gi1", gi1, I32), ("dl1", dl1, F32),
                                 ("wa1", wa1, F32), ("gi2", gi2, I32),
                                 ("dl2", dl2, F32), ("wa2", wa2, F32)):
                t = cpool.tile(list(ap.shape), dt, name=f"m_{name}")
                nc.sync.dma_start(out=t[:], in_=ap[:])
                meta_t[name] = t

            W = {}
            for n in wnames:
                t = cpool.tile(wshapes[n], BF16, name=f"c_{n}")
                nc.sync.dma_start(out=t[:], in_=wap[n][:])
                W[n] = t
            for n in bnames:
                t = cpool.tile([P, H], F32, name=f"c_{n}")
                nc.sync.dma_start(out=t[:], in_=bap[n][:])
                W[n] = t
            wl2_t = cpool.tile([P, KH * H], BF16, name="wl2_t")
            nc.sync.dma_start(out=wl2_t[:], in_=wl2[:])
            wr2_t = cpool.tile([P, KH * H], BF16, name="wr2_t")
            nc.sync.dma_start(out=wr2_t[:], in_=wr2[:])
            wcls_t = cpool.tile([P, KH * CLS], BF16, name="wcls_t")
            nc.sync.dma_start(out=wcls_t[:], in_=wcls[:])
            bcls_t = cpool.tile([P, CLS], F32, name="bcls_t")
            nc.sync.dma_start(out=bcls_t[:], in_=bcls_bc[:])
            zero_row = cpool.tile([P, 2 * H], BF16)
            nc.vector.memset(zero_row[:], 0.0)

            def ag(quarters, full, q):
                nc.gpsimd.collective_compute(
                    "AllGather", OP.bypass,
                    replica_groups=[list(range(NCORES))],
                    ins=[quarters[q][:].opt()], outs=[full[q][:].opt()])

            def transposes(src_sb, psum_pool, sbuf_pool, nchunks, tag):
                """PE-transpose bf16 [128, n*128] -> list of [128,128] tiles."""
                outs = []
                for k in range(nchunks):
                    ps = psum_pool.tile([P, P], BF16, name=f"tp_{tag}",
                                        tag="small", space="PSUM")
                    nc.tensor.transpose(out=ps[:], in_=src_sb[:, k * P:(k + 1) * P],
                                        identity=ident[:])
                    sb = sbuf_pool.tile([P, P], BF16, name=f"ts_{tag}{k}",
                                        tag=f"ts_{tag}{k}")
                    nc.vector.tensor_copy(out=sb[:], in_=ps[:])
                    outs.append(sb)
                return outs

            def layernorm(x_sb, pool, tag):
                """returns xn tile fp32 = (x - mean)/sqrt(var+eps)."""
                s1 = pool.tile([P, 1], F32, name=f"{tag}s1", tag=f"{tag}s1")
                nc.vector.reduce_sum(out=s1[:], in_=x_sb[:], axis=AX)
                sq = pool.tile([P, H], F32, name=f"{tag}sq", tag=f"{tag}sq")
                ssq = pool.tile([P, 1], F32, name=f"{tag}ssq", tag=f"{tag}ssq")
                nc.scalar.activation(out=sq[:], in_=x_sb[:], func=AF.Square,
                                     accum_out=ssq[:])
                mu = pool.tile([P, 1], F32, name=f"{tag}mu", tag=f"{tag}mu")
                nc.vector.tensor_scalar_mul(out=mu[:], in0=s1[:], scalar1=1.0 / H)
                var = pool.tile([P, 1], F32, name=f"{tag}var", tag=f"{tag}var")
                nc.vector.tensor_scalar_mul(out=var[:], in0=ssq[:], scalar1=1.0 / H)
                mu2 = pool.tile([P, 1], F32, name=f"{tag}mu2", tag=f"{tag}mu2")
                nc.vector.tensor_tensor(out=mu2[:], in0=mu[:], in1=mu[:], op=OP.mult)
                nc.vector.tensor_tensor(out=var[:], in0=var[:], in1=mu2[:],
                                        op=OP.subtract)
                nc.vector.tensor_scalar_add(out=var[:], in0=var[:], scalar1=EPS)
                rv = pool.tile([P, 1], F32, name=f"{tag}rv", tag=f"{tag}rv")
                nc.vector.reciprocal(out=rv[:], in_=var[:])
                rstd = pool.tile([P, 1], F32, name=f"{tag}rstd", tag=f"{tag}rstd")
                nc.scalar.activation(out=rstd[:], in_=rv[:], func=AF.Sqrt)
                xn = pool.tile([P, H], F32, name=f"{tag}xn", tag=f"{tag}xn")
                nc.vector.tensor_scalar(out=xn[:], in0=x_sb[:], scalar1=mu[:],
                                        scalar2=rstd[:], op0=OP.subtract,
                                        op1=OP.mult)
                return xn

            # ================= FUSE + layer-1 hl/hr =================
            with (
                tc.tile_pool(name="fsb", bufs=2) as fsb,
                tc.tile_pool(name="fps", bufs=2, space="PSUM") as fps,
            ):
                def fuse_block(b):
                    sfx = "p" if b < NB_P else "g"
                    bl = b if b < NB_P else b - NB_P          # block in its type
                    shrow = bl * P if b < NB_P else PSH + bl * P  # shard row
                    q, lo = divmod(shrow, QROWS)
                    xmt_src, xet_src = (xmt_p, xet_p) if b < NB_P else (xmt_g, xet_g)
                    xmT = fsb.tile([P, KM * P], BF16, tag="xmT", name="xmT")
                    xeT = fsb.tile([P, KE * P], BF16, tag="xeT", name="xeT")
                    nc.sync.dma_start(out=xmT[:],
                                      in_=xmt_src[:, bl * KM * P:(bl + 1) * KM * P])
                    nc.sync.dma_start(out=xeT[:],
                                      in_=xet_src[:, bl * KE * P:(bl + 1) * KE * P])
                    hps = fps.tile([P, H], F32, tag="hps", space="PSUM", name="hps")
                    for k in range(KM):
                        nc.tensor.matmul(out=hps[:, :H // 2],
                                         lhsT=xmT[:, k * P:(k + 1) * P],
                                         rhs=W[f"wm_{sfx}"][:, k * (H // 2):(k + 1) * (H // 2)],
                                         start=(k == 0), stop=(k == KM - 1))
                    for k in range(KE):
                        nc.tensor.matmul(out=hps[:, H // 2:],
                                         lhsT=xeT[:, k * P:(k + 1) * P],
                                         rhs=W[f"we_{sfx}"][:, k * (H // 2):(k + 1) * (H // 2)],
                                         start=(k == 0), stop=(k == KE - 1))
                    cat = fsb.tile([P, H], F32, tag="cat", name="cat")
                    nc.vector.tensor_tensor(out=cat[:], in0=hps[:],
                                            in1=W[f"bfuse_{sfx}"][:], op=OP.add)
                    xn = layernorm(cat, fsb, "f_")
                    h1a = fsb.tile([P, H], F32, tag="h1a", name="h1a")
                    nc.vector.tensor_tensor(out=h1a[:], in0=xn[:],
                                            in1=W[f"gbc_{sfx}"][:], op=OP.mult)
                    h1 = fsb.tile([P, H], BF16, tag="h1", name="h1")
                    nc.vector.tensor_tensor(out=h1[:], in0=h1a[:],
                                            in1=W[f"bbc_{sfx}"][:], op=OP.add)
                    nc.sync.dma_start(out=xh1_q[q][lo:lo + P, 0:H], in_=h1[:])
                    xnb = fsb.tile([P, H], BF16, tag="xnb", name="xnb")
                    nc.scalar.activation(out=xnb[:], in_=xn[:], func=AF.Copy)
                    xnT = transposes(xnb, fps, fsb, KH, "xn")
                    for wnm, bnm, dst_ap in (
                        ("wl1", "bl1", xh1_q[q][lo:lo + P, H:2 * H]),
                        ("wr1", "br1", hr1_tab[shrow:shrow + P, :]),
                    ):
                        ps = fps.tile([P, H], F32, tag="hlr", space="PSUM",
                                      name="hlr")
                        for k in range(KH):
                            nc.tensor.matmul(out=ps[:], lhsT=xnT[k][:],
                                             rhs=W[f"{wnm}_{sfx}"][:, k * H:(k + 1) * H],
                                             start=(k == 0), stop=(k == KH - 1))
                        sb = fsb.tile([P, H], BF16, tag="hlr_sb", name="hlr_sb")
                        nc.vector.tensor_tensor(out=sb[:], in0=ps[:],
                                                in1=W[f"{bnm}_{sfx}"][:], op=OP.add)
                        nc.sync.dma_start(out=dst_ap, in_=sb[:])

                def zero_pads_p():
                    # p pad rows [5000:5120] live in quarter 2
                    q, lo = divmod(PPC, QROWS)
                    n = PSH - PPC
                    nc.sync.dma_start(out=xh1_q[q][lo:lo + n, :],
                                      in_=zero_row[:n, :])
                    nc.sync.dma_start(out=hr1_tab[PPC:PSH, :],
                                      in_=zero_row[:n, :H])

                def zero_pads_g():
                    # g pad rows [7620:7680] live in quarter 3
                    q, lo = divmod(PSH + GPC, QROWS)
                    n = GSH - GPC
                    nc.sync.dma_start(out=xh1_q[q][lo:lo + n, :],
                                      in_=zero_row[:n, :])
                    nc.sync.dma_start(out=hr1_tab[PSH + GPC:SH, :],
                                      in_=zero_row[:n, :H])

                for b in range(30):                     # rows 0..3840 (q0, q1)
                    fuse_block(b)
                    if b == 14:
                        ag(xh1_q, xh1_f, 0)
                    elif b == 29:
                        ag(xh1_q, xh1_f, 1)
                for b in range(30, 40):                 # p rows 3840..5120
                    fuse_block(b)
                zero_pads_p()
                for b in range(NB_P, NB_P + 5):         # g rows 5120..5760
                    fuse_block(b)
                ag(xh1_q, xh1_f, 2)
                for b in range(NB_P + 5, NB_P + NB_G):  # g rows 5760..7680
                    fuse_block(b)
                zero_pads_g()
                ag(xh1_q, xh1_f, 3)

            # ================= edge phase (both layers) =================
            def edge_phase(layer, xh_f, hr_tab, nblocks, gi, dl, wa,
                           om_ap, epilogue, ag_after=None):
                with (
                    tc.tile_pool(name=f"esb{layer}", bufs=2) as esb,
                    tc.tile_pool(name=f"exs{layer}", bufs=2) as exs,
                    tc.tile_pool(name=f"eps{layer}", bufs=2, space="PSUM") as eps,
                    tc.tile_pool(name=f"exo{layer}", bufs=1, space="PSUM") as exo,
                ):
                    for b in range(nblocks):
                        col0 = b * CB
                        shrow = b * P if b < NB_P else PSH + (b - NB_P) * P
                        hr_sb = esb.tile([P, H], BF16, tag="hr_sb", name="hr_sb")
                        nc.sync.dma_start(out=hr_sb[:],
                                          in_=hr_tab[shrow:shrow + P, :])
                        xs = exs.tile([P, CB * 2 * H], BF16, tag="xs", name="xs")
                        for j in range(CB):
                            nc.gpsimd.indirect_dma_start(
                                out=xs[:, j * 2 * H:(j + 1) * 2 * H],
                                out_offset=None, in_=xh_f[j // CQ][:],
                                in_offset=bass.IndirectOffsetOnAxis(
                                    ap=meta_t[gi][:, col0 + j:col0 + j + 1],
                                    axis=0))
                        om_blk = esb.tile([P, CB * P], BF16, tag="om",
                                          name="om_blk")
                        nc.sync.dma_start(out=om_blk[:],
                                          in_=om_ap[:, col0 * P:(col0 + CB) * P])
                        oh_all = esb.tile([P, CB * P], BF16, tag="oh_all",
                                          name="oh_all")
                        eraw = esb.tile([P, CB], F32, tag="eraw", name="eraw")
                        for j in range(CB):
                            he = eps.tile([P, H], F32, tag="big", space="PSUM",
                                          name="he")
                            nc.tensor.matmul(out=he[:],
                                             lhsT=om_blk[:, j * P:(j + 1) * P],
                                             rhs=hr_sb[:], start=True, stop=True)
                            tb = esb.tile([P, H], F32, tag="tb", name="tb")
                            nc.vector.tensor_tensor(
                                out=tb[:], in0=xs[:, j * 2 * H + H:(j + 1) * 2 * H],
                                in1=he[:], op=OP.add)
                            lk = esb.tile([P, H], BF16, tag="lk", name="lk")
                            nc.scalar.activation(out=lk[:], in_=tb[:],
                                                 func=AF.Prelu, alpha=SLOPE,
                                                 accum_out=eraw[:, j:j + 1])
                            nc.vector.tensor_tensor(
                                out=oh_all[:, j * P:(j + 1) * P],
                                in0=meta_t[dl][:, col0 + j:col0 + j + 1].to_broadcast([P, P]),
                                in1=iota[:], op=OP.is_equal)
                        e_all = esb.tile([P, CB], F32, tag="e_all", name="e_all")
                        nc.vector.tensor_tensor(out=e_all[:],
                                                in0=eraw[:],
                                                in1=meta_t[wa][:, col0:col0 + CB],
                                                op=OP.mult)
                        cm_all = esb.tile([P, CB], F32, tag="cm_all",
                                          name="cm_all")
                        for j in range(CB):
                            m2 = esb.tile([P, P], BF16, tag="m2", name="m2")
                            nc.vector.tensor_scalar(
                                out=m2[:], in0=oh_all[:, j * P:(j + 1) * P],
                                scalar1=e_all[:, j:j + 1], scalar2=None,
                                op0=OP.mult)
                            m2T = eps.tile([P, P], BF16, tag="small",
                                           space="PSUM", name="m2T")
                            nc.tensor.transpose(out=m2T[:], in_=m2[:],
                                                identity=ident[:])
                            nc.vector.tensor_reduce(out=cm_all[:, j:j + 1],
                                                    in_=m2T[:], axis=AX,
                                                    op=OP.max)
                        kmax = esb.tile([P, 1], F32, tag="kmax", name="kmax")
                        nc.vector.tensor_reduce(out=kmax[:], in_=cm_all[:],
                                                axis=AX, op=OP.max)
                        nc.vector.tensor_scalar_max(out=kmax[:], in0=kmax[:],
                                                    scalar1=0.0)
                        negkb = esb.tile([P, 1], BF16, tag="negkb", name="negkb")
                        nc.vector.tensor_scalar_mul(out=negkb[:], in0=kmax[:],
                                                    scalar1=-1.0)
                        pcols = exo.tile([P, CB], F32, tag="pcols",
                                         space="PSUM", name="pcols")
                        for j in range(CB):
                            nc.tensor.matmul(out=pcols[:, j:j + 1],
                                             lhsT=om_blk[:, j * P:(j + 1) * P],
                                             rhs=negkb[:], start=True, stop=True)
                        ue = esb.tile([P, CB], F32, tag="ue", name="ue")
                        nc.vector.tensor_tensor(out=ue[:], in0=e_all[:],
                                                in1=pcols[:], op=OP.add)
                        u_all = esb.tile([P, CB], F32, tag="u_all",
                                         name="u_all")
                        nc.scalar.activation(out=u_all[:], in_=ue[:],
                                             func=AF.Exp)
                        u_bf = esb.tile([P, CB], BF16, tag="u_bf", name="u_bf")
                        nc.vector.tensor_copy(out=u_bf[:], in_=u_all[:])
                        den = exo.tile([P, 1], F32, tag="den", space="PSUM",
                                       name="den")
                        for j in range(CB):
                            nc.tensor.matmul(out=den[:],
                                             lhsT=oh_all[:, j * P:(j + 1) * P],
                                             rhs=u_bf[:, j:j + 1],
                                             start=(j == 0), stop=(j == CB - 1))
                        den_sb = esb.tile([P, 1], F32, tag="den_sb",
                                          name="den_sb")
                        nc.vector.tensor_scalar_max(out=den_sb[:], in0=den[:],
                                                    scalar1=1e-30)
                        rden = esb.tile([P, 1], F32, tag="rden", name="rden")
                        nc.vector.reciprocal(out=rden[:], in_=den_sb[:])
                        xo_ps = exo.tile([P, H], F32, tag="xo", space="PSUM",
                                         name="xo")
                        for j in range(CB):
                            ohu = esb.tile([P, P], BF16, tag="ohu", name="ohu")
                            nc.vector.tensor_scalar(
                                out=ohu[:], in0=oh_all[:, j * P:(j + 1) * P],
                                scalar1=u_all[:, j:j + 1], scalar2=None,
                                op0=OP.mult)
                            nc.tensor.matmul(out=xo_ps[:], lhsT=ohu[:],
                                             rhs=xs[:, j * 2 * H:j * 2 * H + H],
                                             start=(j == 0), stop=(j == CB - 1))
                        # xo = (sum_e u_e x_e) / den  -- exact per-row scale
                        xo_sb = esb.tile([P, H], F32, tag="xo_sb", name="xo_sb")
                        nc.vector.tensor_scalar(out=xo_sb[:], in0=xo_ps[:],
                                                scalar1=rden[:], scalar2=None,
                                                op0=OP.mult)
                        epilogue(b, xo_sb, esb, eps)
                        if ag_after and b in ag_after:
                            ag(xh2_q, xh2_f, ag_after[b])

            # ---- layer 1 ----
            def epi1(b, xo_sb, esb, eps):
                shrow = b * P if b < NB_P else PSH + (b - NB_P) * P
                q, lo = divmod(shrow, QROWS)
                xob = esb.tile([P, H], BF16, tag="xob", name="xob")
                nc.scalar.activation(out=xob[:], in_=xo_sb[:], func=AF.Copy)
                nc.sync.dma_start(out=xh2_q[q][lo:lo + P, 0:H], in_=xob[:])
                xoT = transposes(xob, eps, esb, KH, "xoT")
                ps = eps.tile([P, H], F32, tag="big", space="PSUM", name="hl2ps")
                for k in range(KH):
                    nc.tensor.matmul(out=ps[:], lhsT=xoT[k][:],
                                     rhs=wl2_t[:, k * H:(k + 1) * H],
                                     start=(k == 0), stop=(k == KH - 1))
                sb = esb.tile([P, H], BF16, tag="hl2sb", name="hl2sb")
                nc.vector.tensor_copy(out=sb[:], in_=ps[:])
                nc.sync.dma_start(out=xh2_q[q][lo:lo + P, H:2 * H], in_=sb[:])
                if b < NB_P:
                    ps2 = eps.tile([P, H], F32, tag="big", space="PSUM",
                                   name="hr2ps")
                    for k in range(KH):
                        nc.tensor.matmul(out=ps2[:], lhsT=xoT[k][:],
                                         rhs=wr2_t[:, k * H:(k + 1) * H],
                                         start=(k == 0), stop=(k == KH - 1))
                    sb2 = esb.tile([P, H], BF16, tag="hr2sb", name="hr2sb")
                    nc.vector.tensor_copy(out=sb2[:], in_=ps2[:])
                    nc.sync.dma_start(out=hr2_tab[shrow:shrow + P, :], in_=sb2[:])

            edge_phase(1, xh1_f, hr1_tab, NB_P + NB_G,
                       "gi1", "dl1", "wa1", om1, epi1,
                       ag_after={14: 0, 29: 1, NB_P + 4: 2, NB_P + NB_G - 1: 3})

            # ---- layer 2 + head LN + cls ----
            def epi2(b, xo_sb, esb, eps):
                xn = layernorm(xo_sb, esb, "c_")
                xnb = esb.tile([P, H], BF16, tag="c_xnb", name="c_xnb")
                nc.scalar.activation(out=xnb[:], in_=xn[:], func=AF.Copy)
                xnT = transposes(xnb, eps, esb, KH, "c_xnT")
                for half in range(2):
                    ps = eps.tile([P, H], F32, tag="big", space="PSUM",
                                  name="clsps")
                    for k in range(KH):
                        nc.tensor.matmul(
                            out=ps[:], lhsT=xnT[k][:],
                            rhs=wcls_t[:, k * CLS + half * H:
                                       k * CLS + (half + 1) * H],
                            start=(k == 0), stop=(k == KH - 1))
                    ob = esb.tile([P, H], F32, tag="c_ob", name="c_ob")
                    nc.vector.tensor_tensor(
                        out=ob[:], in0=ps[:],
                        in1=bcls_t[:, half * H:(half + 1) * H], op=OP.add)
                    nc.sync.dma_start(
                        out=out_d[b * P:(b + 1) * P, half * H:(half + 1) * H],
                        in_=ob[:])

            edge_phase(2, xh2_f, hr2_tab, NB_P, "gi2", "dl2", "wa2", om2, epi2)

    nc.compile()
    return nc


def prepare(inputs):
    """Build program + per-core input maps. Returns (nc, in_maps)."""
    inp = {k: np.asarray(v) for k, v in inputs.items()}

    def rel_arrays(st, dt, s, d):
        nt = NpR if dt == 0 else NgR
        ideg = np.bincount(d, minlength=nt)
        w = (1.0 / np.maximum(ideg[d], 1)).astype(np.float32)
        core, loc = _dst_local(d, dt == 1)
        return (_src_glob(s, st == 1), core, loc, w)

    rels = [(0, 0, inp["pp_src"], inp["pp_dst"]),
            (0, 1, inp["pg_src"], inp["pg_dst"]),
            (1, 0, inp["gp_src"], inp["gp_dst"]),
            (1, 1, inp["gg_src"], inp["gg_dst"])]
    rel1 = [rel_arrays(*r) for r in rels]
    meta1 = _build_meta(rel1, NB_G)
    rel2 = [rel_arrays(*r) for r in rels if r[1] == 0]
    meta2 = _build_meta(rel2, 0)

    lnf_p_g, lnf_p_b = inp["lnf_p_g"], inp["lnf_p_b"]
    lnf_g_g, lnf_g_b = inp["lnf_g_g"], inp["lnf_g_b"]
    wl1 = np.ascontiguousarray(inp["node_Wl"][0].T, dtype=np.float32)
    wr1 = np.ascontiguousarray(inp["node_Wr"][0].T, dtype=np.float32)
    wl2 = np.ascontiguousarray(inp["node_Wl"][1].T, dtype=np.float32)
    wr2 = np.ascontiguousarray(inp["node_Wr"][1].T, dtype=np.float32)
    hg, hb = inp["head_ln_g"], inp["head_ln_b"]
    wcls_T = np.ascontiguousarray(inp["Wcls"].T, dtype=np.float32)

    common = dict(
        wm_p=_kn(np.ascontiguousarray(inp["Wmsa_p"].T, np.float32)),
        wm_g=_kn(np.ascontiguousarray(inp["Wmsa_g"].T, np.float32)),
        we_p=_kn(np.ascontiguousarray(inp["Wesm_p"].T, np.float32)),
        we_g=_kn(np.ascontiguousarray(inp["Wesm_g"].T, np.float32)),
        bfuse_p=_bc(np.concatenate([inp["bmsa_p"], np.zeros(H // 2)])),
        bfuse_g=_bc(np.concatenate([inp["bmsa_g"], np.zeros(H // 2)])),
        gbc_p=_bc(lnf_p_g), bbc_p=_bc(lnf_p_b),
        gbc_g=_bc(lnf_g_g), bbc_g=_bc(lnf_g_b),
        wl1_p=_kn(lnf_p_g[:, None] * wl1),
        wl1_g=_kn(lnf_g_g[:, None] * wl1),
        wr1_p=_kn(lnf_p_g[:, None] * wr1),
        wr1_g=_kn(lnf_g_g[:, None] * wr1),
        bl1_p=_bc(lnf_p_b @ wl1), bl1_g=_bc(lnf_g_b @ wl1),
        br1_p=_bc(lnf_p_b @ wr1), br1_g=_bc(lnf_g_b @ wr1),
        wl2=_kn(wl2), wr2=_kn(wr2),
        wcls=_kn(hg[:, None] * wcls_T),
        bcls_bc=_bc(np.asarray(inp["bcls"]) + hb @ wcls_T),
        iota_in=np.ascontiguousarray(
            np.tile(np.arange(P, dtype=np.float32)[None, :], (P, 1))),
    )

    in_maps = []
    for c in range(NCORES):
        m = dict(common)
        m["xmt_p"] = _xT(inp["xp_msa"][c * PPC:(c + 1) * PPC], NB_P, KM)
        m["xet_p"] = _xT(inp["xp_esm"][c * PPC:(c + 1) * PPC], NB_P, KE)
        m["xmt_g"] = _xT(inp["xg_msa"][c * GPC:(c + 1) * GPC], NB_G, KM)
        m["xet_g"] = _xT(inp["xg_esm"][c * GPC:(c + 1) * GPC], NB_G, KE)
        m["gi1"], m["dl1"], m["wa1"], m["om1"] = (
            meta1[c]["g_idx"], meta1[c]["d_loc"], meta1[c]["w_a"],
            meta1[c]["ohme"])
        m["gi2"], m["dl2"], m["wa2"], m["om2"] = (
            meta2[c]["g_idx"], meta2[c]["d_loc"], meta2[c]["w_a"],
            meta2[c]["ohme"])
        in_maps.append(m)

    nc = build_program()
    return nc, in_maps


def kernel(_run_kwargs=None, **inputs):
    run_kwargs = _run_kwargs or {}
    nc, in_maps = prepare(inputs)
    res = run_bass_kernel_spmd(nc, in_maps, core_ids=list(range(NCORES)),
                               **run_kwargs)
    out = np.concatenate([res.results[c]["out"][:PPC] for c in range(NCORES)], 0)
    if run_kwargs:
        return np.ascontiguousarray(out, np.float32), res
    return np.ascontiguousarray(out, np.float32)


if __name__ == "__main__":
    import reference
    inputs = {k: np.asarray(v) for k, v in reference.setup_inputs().items()}
    got = kernel(**inputs)
    print("kernel output", got.shape, got.dtype, "finite:", np.isfinite(got).all())
